# revision 2
# baseline (speedup 1.0000x reference)
"""CumAvgPool1d Trainium2 kernel.

y[b, c, t] = mean(x[b, c, :t+1]) = cumsum(x, -1)[b, c, t] / (t+1)

Full input x: [8, 512, 16384] f32. Sharding: batch dim across the 8
NeuronCores (core i gets batch i -> [512, 16384] per core, no
communication; cumsum runs along the unsharded time axis).

Per-core plan (memory-bound target):
  - channels on SBUF partitions (4 blocks of 128), time on the free axis
  - time tiled at 4096 (2 MiB f32 DMAs -> near-peak HBM streaming)
  - cumsum along free axis with VectorE tensor_tensor_scan (op0=add,
    op1=bypass), chained across time tiles via initial=prev[:, -1:]
  - divide == multiply by 1/(t+1): host passes invc[1, 16384]; broadcast
    once to [128, 16384] on-chip (gpsimd partition_broadcast), then one
    fp32 tensor_mul per tile
  - loads on nc.sync (HWDGE/SP ring), stores on nc.scalar (HWDGE/ACT
    ring) so the two streams ride separate descriptor rings
"""

import sys

sys.path.insert(0, "/opt/trn_rl_repo")

import numpy as np

B, C, T = 8, 512, 16384
CB = 128  # channel block = SBUF partitions
TT = 4096  # time tile (free axis)
N_CB = C // CB
N_TT = T // TT
N_CORES = 8

_PROGRAM = None


def _build_program():
    from concourse import bacc, mybir
    from concourse.tile import TileContext

    nc = bacc.Bacc(
        "TRN2", target_bir_lowering=False, debug=False, num_devices=N_CORES
    )
    f32 = mybir.dt.float32
    x = nc.dram_tensor("x", [C, T], f32, kind="ExternalInput")
    invc = nc.dram_tensor("invc", [1, T], f32, kind="ExternalInput")
    y = nc.dram_tensor("y", [C, T], f32, kind="ExternalOutput")
    add = mybir.AluOpType.add
    bypass = mybir.AluOpType.bypass

    with TileContext(nc) as tc:
        with (
            tc.tile_pool(name="const", bufs=1) as cpool,
            tc.tile_pool(name="io", bufs=2) as pool,
        ):
            # Resident 1/(t+1) row replicated across all 128 partitions.
            inv_sb = cpool.tile([CB, T], f32, tag="inv")
            for k in range(N_TT):
                stage = cpool.tile([1, TT], f32, tag="stage")
                nc.sync.dma_start(out=stage, in_=invc.ap()[0:1, k * TT : (k + 1) * TT])
                nc.gpsimd.partition_broadcast(
                    inv_sb[:, k * TT : (k + 1) * TT], stage
                )

            for cb in range(N_CB):
                rows = slice(cb * CB, (cb + 1) * CB)
                prev = None
                for t in range(N_TT):
                    cols = slice(t * TT, (t + 1) * TT)
                    it = pool.tile([CB, TT], f32, tag="in")
                    nc.sync.dma_start(out=it, in_=x.ap()[rows, cols])
                    cum = pool.tile([CB, TT], f32, tag="cum")
                    init = 0.0 if prev is None else prev[:, TT - 1 : TT]
                    nc.vector.tensor_tensor_scan(
                        out=cum, data0=it, data1=it, initial=init, op0=add, op1=bypass
                    )
                    ot = pool.tile([CB, TT], f32, tag="out")
                    nc.vector.tensor_mul(
                        out=ot, in0=cum, in1=inv_sb[:, cols]
                    )
                    nc.scalar.dma_start(out=y.ap()[rows, cols], in_=ot)
                    prev = cum
    nc.compile()
    return nc


def _get_program():
    global _PROGRAM
    if _PROGRAM is None:
        _PROGRAM = _build_program()
    return _PROGRAM


def _run(x, trace=False):
    from concourse.bass_utils import run_bass_kernel_spmd

    x = np.ascontiguousarray(np.asarray(x, dtype=np.float32))
    assert x.shape == (B, C, T), x.shape
    inv = (np.float32(1.0) / np.arange(1, T + 1, dtype=np.float32)).reshape(1, T)
    in_maps = [
        {"x": np.ascontiguousarray(x[i]), "invc": inv} for i in range(N_CORES)
    ]
    nc = _get_program()
    bkr = run_bass_kernel_spmd(
        nc, in_maps, core_ids=list(range(N_CORES)), trace=trace
    )
    out = np.stack([r["y"] for r in bkr.results], axis=0)
    return out.astype(np.float32), bkr


def kernel(x):
    out, _ = _run(x, trace=False)
    return out


def run_traced(x):
    """test.py helper: returns (output, BassKernelResults with exec_time_ns)."""
    return _run(x, trace=True)
